# revision 10
# baseline (speedup 1.0000x reference)
"""CenterLoss kernel for Trainium2 (Bass/Tile), data-parallel over 8 NeuronCores.

loss = 0.5 * sum_i ||x_i - centers[targets_i]||^2

The reference materializes the full [N, C] distance matrix and gathers one
entry per row; here we gather only the target center rows (indirect DMA) and
fuse subtract/square/row-reduce into a single custom DVE op, so the kernel is
memory-bound on the gathered traffic instead of a 69 GFLOP matmul.

Sharding: inputs/targets split along batch N across 8 cores (512 rows each),
centers replicated. Each core partition-reduces its partials on the (idle)
PE and returns a handful of scalars; the host sums them and scales by 0.5.

The device computes in bf16 (inputs/centers are cast on the host while
sharding): the loss is a sum of 4M positive squared terms, so bf16 rounding
gives ~1e-3 relative error against the 2e-2 tolerance, while halving the
HBM traffic that the 16 DMA engines (~26 GB/s each, at the HBM roofline)
are bottlenecked on. Accumulation is fp32 on the DVE accumulator and fp64
on the host.

Compute uses a custom ant-DVE op (registered below): one Vector pass per
1024-column block computes (x - c)^2 and row-reduces it into an fp32
accumulator column — no ACT activations, no accumulator-read instructions.
"""

import numpy as np
import ml_dtypes

import concourse.bacc as bacc
import concourse.bass as bass
import concourse.tile as tile
from concourse import mybir
from concourse import dve_ops
from concourse.dve_spec import C0, Spec, Src0, Src1, sq
from concourse.dve_uop import DveOpSpec
from concourse.bass_utils import run_bass_kernel_spmd

N, C, D = 4096, 8192, 1024
N_CORES = 8
ROWS = N // N_CORES  # 512 rows per core
P = 128              # SBUF partitions
CHUNKS = ROWS // P   # 4 idx columns of 128 rows

# Gather split: one 256-row indirect DMA (SWDGE fixed cost is ~1 us per
# instruction) then two 128-row ones so the final chunk's completion -> tail
# chain is short.
GATHERS = [2, 1, 1]  # idx columns per indirect DMA

NACC = 5  # accum cols: col0, col1, col2, col3 in two half-width slices

# Stashed BassKernelResults from the most recent kernel() call (for profiling).
LAST_RESULTS = None
_NC_CACHE = None


def _sqdiff_reduce_op() -> dve_ops.DveOp:
    """Register (once) and return the fused op:

        out[k]    = (in0[k] - in1[k])^2
        accum_out = c0 + sum_k out[k]

    Registered into concourse.dve_ops.OPS at import time (the documented
    extension point; done at runtime because the repo is read-only). The
    uops sha is self-pinned from lower()'s output; correctness is checked
    end-to-end against the fp32 reference.
    """
    name = "CENTERLOSS_SQDIFF_REDUCE"
    for op in dve_ops.OPS:
        if op.name == name:
            return op
    spec = Spec(
        body=sq(Src0 - Src1),
        accum=__import__("operator").add,
        accum_init=C0,
        reference=dve_ops._ref_body_sum(
            lambda in0, in1, c0, c1, c2: (in0.astype(np.float32) - in1) ** 2
        ),
    )
    shas = {}
    for ver in ("v3", "v4"):
        s = DveOpSpec(name=name, opcode=0, uops=dve_ops.lower(spec, ver=ver),
                      rd1_en=True)
        shas[ver] = s.sha(ver)
    op = dve_ops.DveOp(name, spec, subdim=False, uops_sha=shas)
    dve_ops.OPS.append(op)
    dve_ops.CUSTOM_DVE_SPECS[name] = spec
    dve_ops._SUB_OPCODE_FOR_NAME[name] = (
        dve_ops._CUSTOM_DVE_ROW_BASE + len(dve_ops.OPS) - 1
    )
    assert dve_ops._SUB_OPCODE_FOR_NAME[name] < 0x20
    return op


_SQDIFF = _sqdiff_reduce_op()


def _build_bass():
    nc = bacc.Bacc("TRN2", target_bir_lowering=False)
    x = nc.dram_tensor("x", [ROWS, D], mybir.dt.bfloat16, kind="ExternalInput")
    idx = nc.dram_tensor("idx", [P, CHUNKS], mybir.dt.int32, kind="ExternalInput")
    centers = nc.dram_tensor("centers", [C, D], mybir.dt.bfloat16, kind="ExternalInput")
    out = nc.dram_tensor("out", [1, NACC], mybir.dt.float32, kind="ExternalOutput")

    with tile.TileContext(nc) as tc:
        with (
            tc.tile_pool(name="io", bufs=1) as io,
            tc.tile_pool(name="cpool", bufs=len(GATHERS)) as cp,
            tc.tile_pool(name="psum", bufs=1, space="PSUM") as pp,
            tc.tile_pool(name="small", bufs=1) as small,
        ):
            # idx and x BOTH on the Scalar (Activation) HWDGE ring, idx
            # first: same-queue FIFO guarantees idx's 128 tiny descriptors
            # complete before the fat x descriptors start. (With idx on the
            # Sync queue it nondeterministically starves behind the x
            # stream for up to ~3 us — the engines drain one hardware
            # queue's backlog before the other's.)
            idx_sb = small.tile([P, CHUNKS], mybir.dt.int32)
            nc.scalar.dma_start(idx_sb[:], idx[:, :])
            # x as ONE 1 MB DMA; partition p holds rows 4p..4p+3 = 8 KB
            # contiguous in DRAM -> 128 fat descriptors.
            x_dram = x.rearrange("(p u) d -> p (u d)", p=P)
            x_sb = io.tile([P, CHUNKS * D], mybir.dt.bfloat16, tag="x")
            nc.scalar.dma_start(x_sb[:], x_dram[:, :])
            ones = small.tile([P, 1], mybir.dt.float32)
            nc.vector.memset(ones[:], 1.0)
            acc = small.tile([P, NACC], mybir.dt.float32)

            col = 0   # current idx column
            a = 0     # current accumulator column
            for gi, gcols in enumerate(GATHERS):
                ct = cp.tile([P, gcols * D], mybir.dt.bfloat16, tag="c")
                nc.gpsimd.indirect_dma_start(
                    out=ct[:],
                    out_offset=None,
                    in_=centers[:, :],
                    in_offset=bass.IndirectOffsetOnAxis(
                        ap=idx_sb[:, col : col + gcols], axis=0
                    ),
                )
                for u in range(gcols):
                    xoff = (col + u) * D
                    last = gi == len(GATHERS) - 1 and u == gcols - 1
                    # Half-width slices on the very last column to shorten
                    # the serial gather -> sqdiff tail.
                    bounds = [(0, D // 2), (D // 2, D)] if last else [(0, D)]
                    for cs, ce in bounds:
                        nc.vector._custom_dve(
                            _SQDIFF,
                            out=ct[:, u * D + cs : u * D + ce],
                            in0=x_sb[:, xoff + cs : xoff + ce],
                            in1=ct[:, u * D + cs : u * D + ce],
                            s0=0.0,
                            accum_out=acc[:, a : a + 1],
                        )
                        a += 1
                col += gcols
            assert a == NACC

            # Partition-reduce on the (idle) PE: ones^T @ acc-cols. The
            # first three cols are reduced and shipped while the tail is
            # still computing; each output DMA is a single small descriptor.
            psum_a = pp.tile([1, 3], mybir.dt.float32, tag="pa")
            nc.tensor.matmul(
                psum_a[:], lhsT=ones[:], rhs=acc[:, :3],
                start=True, stop=True,
            )
            res_a = small.tile([1, 3], mybir.dt.float32)
            nc.vector.tensor_copy(res_a[:], psum_a[:])
            nc.sync.dma_start(out[:, :3], res_a[:])
            psum_b = pp.tile([1, NACC - 3], mybir.dt.float32, tag="pb")
            nc.tensor.matmul(
                psum_b[:], lhsT=ones[:], rhs=acc[:, 3:],
                start=True, stop=True,
            )
            res_b = small.tile([1, NACC - 3], mybir.dt.float32)
            nc.vector.tensor_copy(res_b[:], psum_b[:])
            nc.sync.dma_start(out[:, 3:], res_b[:])
    nc.finalize()
    return nc


def _get_nc():
    global _NC_CACHE
    if _NC_CACHE is None:
        _NC_CACHE = _build_bass()
    return _NC_CACHE


def kernel(inputs, targets, centers):
    global LAST_RESULTS
    x = np.asarray(inputs, dtype=np.float32).astype(ml_dtypes.bfloat16)
    tgt = np.asarray(targets).astype(np.int32)
    cen = np.ascontiguousarray(
        np.asarray(centers, dtype=np.float32).astype(ml_dtypes.bfloat16)
    )
    assert x.shape == (N, D) and cen.shape == (C, D) and tgt.shape == (N,)

    nc = _get_nc()
    in_maps = []
    for c in range(N_CORES):
        xs = np.ascontiguousarray(x[c * ROWS : (c + 1) * ROWS])
        # idx[p, t] = target of shard row p*CHUNKS + t
        idxs = np.ascontiguousarray(tgt[c * ROWS : (c + 1) * ROWS].reshape(P, CHUNKS))
        in_maps.append({"x": xs, "idx": idxs, "centers": cen})

    res = run_bass_kernel_spmd(nc, in_maps, core_ids=list(range(N_CORES)))
    LAST_RESULTS = res

    total = 0.0
    for r in res.results:
        total += float(r["out"].astype(np.float64).sum())
    return np.array(0.5 * total, dtype=np.float32)


# revision 13
# speedup vs baseline: 1.0742x; 1.0742x over previous
"""CenterLoss kernel for Trainium2 (Bass/Tile), data-parallel over 8 NeuronCores.

loss = 0.5 * sum_i ||x_i - centers[targets_i]||^2

The reference materializes the full [N, C] distance matrix and gathers one
entry per row; here we gather only the target center rows (indirect DMA) and
fuse subtract/square/row-reduce into a single custom DVE op, so the kernel is
memory-bound on the gathered traffic instead of a 69 GFLOP matmul.

Sharding: inputs/targets split along batch N across 8 cores (512 rows each),
centers replicated. Each core partition-reduces its partials on the (idle)
PE and returns a handful of scalars; the host sums them and scales by 0.5.

The device computes in bf16 (inputs/centers are cast on the host while
sharding): the loss is a sum of 4M positive squared terms, so bf16 rounding
gives ~1e-3 relative error against the 2e-2 tolerance, while halving the
HBM traffic that the 16 DMA engines (~26 GB/s each, at the HBM roofline)
are bottlenecked on. Accumulation is fp32 on the DVE accumulator and fp64
on the host.

Compute uses a custom ant-DVE op (registered below): one Vector pass per
1024-column block computes (x - c)^2 and row-reduces it into an fp32
accumulator column — no ACT activations, no accumulator-read instructions.
"""

import numpy as np
import ml_dtypes

import concourse.bacc as bacc
import concourse.bass as bass
import concourse.tile as tile
from concourse import mybir
from concourse import dve_ops
from concourse.dve_spec import C0, Spec, Src0, Src1, sq
from concourse.dve_uop import DveOpSpec
from concourse.bass_utils import run_bass_kernel_spmd

N, C, D = 4096, 8192, 1024
N_CORES = 8
ROWS = N // N_CORES  # 512 rows per core
P = 128              # SBUF partitions
CHUNKS = ROWS // P   # 4 idx columns of 128 rows

# One 128-row indirect DMA per idx column: the first gather completes
# ~1.5 us earlier than with fused 256-row gathers, and the compute chain
# (the critical path) starts at first-gather-complete + sem-prop. The
# longer SWDGE generation chain (~1.4 us fixed per instruction) hides
# under the earlier compute.
GATHERS = [1, 1, 1, 1]  # idx columns per indirect DMA

NACC = 4  # one accumulator column per idx column

# Stashed BassKernelResults from the most recent kernel() call (for profiling).
LAST_RESULTS = None
_NC_CACHE = None


def _sqdiff_reduce_op() -> dve_ops.DveOp:
    """Register (once) and return the fused op:

        out[k]    = (in0[k] - in1[k])^2
        accum_out = c0 + sum_k out[k]

    Registered into concourse.dve_ops.OPS at import time (the documented
    extension point; done at runtime because the repo is read-only). The
    uops sha is self-pinned from lower()'s output; correctness is checked
    end-to-end against the fp32 reference.
    """
    name = "CENTERLOSS_SQDIFF_REDUCE"
    for op in dve_ops.OPS:
        if op.name == name:
            return op
    spec = Spec(
        body=sq(Src0 - Src1),
        accum=__import__("operator").add,
        accum_init=C0,
        reference=dve_ops._ref_body_sum(
            lambda in0, in1, c0, c1, c2: (in0.astype(np.float32) - in1) ** 2
        ),
    )
    shas = {}
    for ver in ("v3", "v4"):
        s = DveOpSpec(name=name, opcode=0, uops=dve_ops.lower(spec, ver=ver),
                      rd1_en=True)
        shas[ver] = s.sha(ver)
    op = dve_ops.DveOp(name, spec, subdim=False, uops_sha=shas)
    dve_ops.OPS.append(op)
    dve_ops.CUSTOM_DVE_SPECS[name] = spec
    dve_ops._SUB_OPCODE_FOR_NAME[name] = (
        dve_ops._CUSTOM_DVE_ROW_BASE + len(dve_ops.OPS) - 1
    )
    assert dve_ops._SUB_OPCODE_FOR_NAME[name] < 0x20
    return op


_SQDIFF = _sqdiff_reduce_op()


def _build_bass():
    nc = bacc.Bacc("TRN2", target_bir_lowering=False)
    x = nc.dram_tensor("x", [ROWS, D], mybir.dt.bfloat16, kind="ExternalInput")
    idx = nc.dram_tensor("idx", [P, CHUNKS], mybir.dt.int32, kind="ExternalInput")
    centers = nc.dram_tensor("centers", [C, D], mybir.dt.bfloat16, kind="ExternalInput")
    out = nc.dram_tensor("out", [1, NACC], mybir.dt.float32, kind="ExternalOutput")

    with tile.TileContext(nc) as tc:
        with (
            tc.tile_pool(name="io", bufs=1) as io,
            tc.tile_pool(name="cpool", bufs=len(GATHERS)) as cp,
            tc.tile_pool(name="psum", bufs=1, space="PSUM") as pp,
            tc.tile_pool(name="small", bufs=1) as small,
        ):
            # idx and x BOTH on the Scalar (Activation) HWDGE ring, idx
            # first: same-queue FIFO guarantees idx's 128 tiny descriptors
            # complete before the fat x descriptors start. (With idx on the
            # Sync queue it nondeterministically starves behind the x
            # stream for up to ~3 us — the engines drain one hardware
            # queue's backlog before the other's.)
            idx_sb = small.tile([P, CHUNKS], mybir.dt.int32)
            nc.scalar.dma_start(idx_sb[:], idx[:, :])
            # x as ONE 1 MB DMA; partition p holds rows 4p..4p+3 = 8 KB
            # contiguous in DRAM -> 128 fat descriptors.
            x_dram = x.rearrange("(p u) d -> p (u d)", p=P)
            x_sb = io.tile([P, CHUNKS * D], mybir.dt.bfloat16, tag="x")
            nc.scalar.dma_start(x_sb[:], x_dram[:, :])
            ones = small.tile([P, 1], mybir.dt.float32)
            nc.vector.memset(ones[:], 1.0)
            # Dummy activation to pull the ACT function-table load off the
            # critical path (blocks 0-1 use ACT squares below).
            warm = small.tile([1, 1], mybir.dt.float32)
            nc.scalar.activation(
                out=warm[:], in_=ones[0:1, :],
                func=mybir.ActivationFunctionType.Square,
            )
            acc = small.tile([P, NACC], mybir.dt.float32)

            # Hybrid compute split across both vector-capable engines: the
            # first two 1024-col blocks go DVE-sub -> ACT-square+accum, the
            # last two are single fused sqdiff+reduce ops on the DVE. Both
            # engines finish within ~0.5 us of each other.
            cts = []
            for col in range(CHUNKS):
                ct = cp.tile([P, D], mybir.dt.bfloat16, tag="c")
                nc.gpsimd.indirect_dma_start(
                    out=ct[:],
                    out_offset=None,
                    in_=centers[:, :],
                    in_offset=bass.IndirectOffsetOnAxis(
                        ap=idx_sb[:, col : col + 1], axis=0
                    ),
                )
                cts.append(ct)
            for col in range(CHUNKS):
                ct = cts[col]
                xs = x_sb[:, col * D : (col + 1) * D]
                if col < 2:
                    nc.vector.tensor_sub(ct[:], xs, ct[:])
                    nc.scalar.activation(
                        out=ct[:],
                        in_=ct[:],
                        func=mybir.ActivationFunctionType.Square,
                        accum_out=acc[:, col : col + 1],
                    )
                else:
                    nc.vector._custom_dve(
                        _SQDIFF,
                        out=ct[:],
                        in0=xs,
                        in1=ct[:],
                        s0=0.0,
                        accum_out=acc[:, col : col + 1],
                    )

            # Partition-reduce on the (idle) PE: ones^T @ acc-cols. The
            # first three cols are reduced and shipped while the tail is
            # still computing; each output DMA is a single small descriptor.
            psum_a = pp.tile([1, 3], mybir.dt.float32, tag="pa")
            nc.tensor.matmul(
                psum_a[:], lhsT=ones[:], rhs=acc[:, :3],
                start=True, stop=True,
            )
            res_a = small.tile([1, 3], mybir.dt.float32)
            nc.vector.tensor_copy(res_a[:], psum_a[:])
            nc.sync.dma_start(out[:, :3], res_a[:])
            psum_b = pp.tile([1, NACC - 3], mybir.dt.float32, tag="pb")
            nc.tensor.matmul(
                psum_b[:], lhsT=ones[:], rhs=acc[:, 3:],
                start=True, stop=True,
            )
            res_b = small.tile([1, NACC - 3], mybir.dt.float32)
            nc.vector.tensor_copy(res_b[:], psum_b[:])
            # out_b from the (idle-by-now) Scalar ring so its trigger does
            # not serialize behind out_a's on Sync.
            nc.scalar.dma_start(out[:, 3:], res_b[:])
    nc.finalize()
    return nc


def _get_nc():
    global _NC_CACHE
    if _NC_CACHE is None:
        _NC_CACHE = _build_bass()
    return _NC_CACHE


def kernel(inputs, targets, centers):
    global LAST_RESULTS
    x = np.asarray(inputs, dtype=np.float32).astype(ml_dtypes.bfloat16)
    tgt = np.asarray(targets).astype(np.int32)
    cen = np.ascontiguousarray(
        np.asarray(centers, dtype=np.float32).astype(ml_dtypes.bfloat16)
    )
    assert x.shape == (N, D) and cen.shape == (C, D) and tgt.shape == (N,)

    nc = _get_nc()
    in_maps = []
    for c in range(N_CORES):
        xs = np.ascontiguousarray(x[c * ROWS : (c + 1) * ROWS])
        # idx[p, t] = target of shard row p*CHUNKS + t
        idxs = np.ascontiguousarray(tgt[c * ROWS : (c + 1) * ROWS].reshape(P, CHUNKS))
        in_maps.append({"x": xs, "idx": idxs, "centers": cen})

    res = run_bass_kernel_spmd(nc, in_maps, core_ids=list(range(N_CORES)))
    LAST_RESULTS = res

    total = 0.0
    for r in res.results:
        total += float(r["out"].astype(np.float64).sum())
    return np.array(0.5 * total, dtype=np.float32)
